# revision 3
# baseline (speedup 1.0000x reference)
"""Trainium2 Bass kernel for ConcreteAttentionModel (dense_mlp).

Model (reference):
  img = relu(einsum('bnf,df->bnd', image_features, W_ic) + b_ic)   B=16,N=64,F=65536,D=512
  gated attention-MIL pooling over patches -> img [B, D]
  text = relu(text @ Wt.T + bt)                                    [B, D]
  h1 = relu(img @ Wh1.T + bh1)
  z1 = einsum('bi,oij,bj->bo', img, Wz1, text) + bz1               Wz1 [D,D,D]
  o1 = relu((sigmoid(z1) * h1) @ Wo1.T + bo1)
  fused = relu(kron(o1, text) @ Wpf.T + bpf)                       Wpf [D, D*D]
  out = fused @ Wcls.T + bcls                                      [B, 1]

Sharding over 8 cores (v2, bf16 data path):
  - Stage 1 sharded over the contraction dim F (8192 per core); bf16 partial
    [D, BN] AllReduce'd on-device; attention pooling replicated per core.
  - Bilinear Wz1 and post-fusion Wpf sharded over their output dim (64 rows
    per core).  Weights stream as bf16 slabs pre-rearranged host-side to
    [o, 128, it*512+j] so every DMA lands 4KB contiguous per partition.
    A single shared FIFO pool of slab buffers lets the wz1 stream prefetch
    during the (PE-bound) stage-1 phase and the wpf stream prefetch during
    the z1 phase.
  - All matmul operands are bf16 (PSUM accumulation fp32); biases fp32.
    Collectives (AllReduce / AllGather) carry bf16 payloads.
"""

import numpy as np
import ml_dtypes

import concourse.bass as bass
import concourse.mybir as mybir
import concourse.tile as tile
from concourse import bacc
from concourse.masks import make_identity

F32 = mybir.dt.float32
BF16 = mybir.dt.bfloat16
NP_BF16 = ml_dtypes.bfloat16

NCORES = 8
B, N, F = 16, 64, 65536
BN = B * N                      # 1024
D, A, T = 512, 256, 18
FS = F // NCORES                # 8192 per-core contraction slice
KT = FS // 128                  # 64 k-tiles
KC = KT // 2                    # 32 k-chunks (2 k-tiles per DMA)
OS = D // NCORES                # 64 output rows per core (bilinear/post-fusion)
DT = D // 128                   # 4 partition tiles of the D dim
AT = A // 128                   # 2 partition tiles of the A dim
NSLAB = 24                      # in-flight big-weight slabs (shared wz1+wpf FIFO)


def build_nc(use_collectives: bool = True, reps: int = 1):
    nc = bacc.Bacc("TRN2", target_bir_lowering=False, debug=False,
                   num_devices=NCORES)

    def dram(name, shape, dt=F32):
        return nc.dram_tensor(name, shape, dt, kind="ExternalInput").ap()

    # per-core sharded inputs, pre-tiled host-side for contiguous DMA
    xT = dram("xT", [KC, 128, 2 * BN], BF16)    # [c, p, k*BN+n]
    wicT = dram("wicT", [KC, 128, 2 * D], BF16)  # [c, p, k*D+d]
    wz1 = dram("wz1", [OS, 128, DT * D], BF16)  # [o, p, it*D+j]
    wpf = dram("wpf", [OS, 128, DT * D], BF16)  # [m, p, it*D+j]
    wclsT = dram("wclsT", [OS, 1], BF16)
    bz1 = dram("bz1", [OS, 1])
    bpf = dram("bpf", [OS, 1])
    # replicated inputs
    textT = dram("textT", [T, B], BF16)
    waT = dram("waT", [D, A], BF16)
    wbT = dram("wbT", [D, A], BF16)
    wcT = dram("wcT", [A, 1], BF16)
    wtT = dram("wtT", [T, D], BF16)
    wh1T = dram("wh1T", [D, D], BF16)
    wo1T = dram("wo1T", [D, D], BF16)
    b_ic = dram("b_ic", [D, 1])
    ba = dram("ba", [A, 1])
    bb = dram("bb", [A, 1])
    bt = dram("bt", [D, 1])
    bh1 = dram("bh1", [D, 1])
    bo1 = dram("bo1", [D, 1])

    out_partial = nc.dram_tensor("out_partial", [1, B], F32,
                                 kind="ExternalOutput").ap()

    groups = [list(range(NCORES))]

    with tile.TileContext(nc) as tc:
        with (
            nc.allow_low_precision(reason="bf16 matmul operands"),
            tc.tile_pool(name="const", bufs=1) as cst,
            tc.tile_pool(name="xk", bufs=6) as xk,
            tc.tile_pool(name="wk", bufs=6) as wk,
            tc.tile_pool(name="wbig", bufs=NSLAB) as wbig,
            tc.tile_pool(name="work", bufs=1) as wrk,
            tc.tile_pool(name="persist", bufs=1) as per,
            tc.tile_pool(name="ps", bufs=8, space="PSUM") as ps,
            tc.tile_pool(name="dramp", bufs=1, space="DRAM") as dp,
        ):
            # ---- constants / small weights -------------------------------
            ident = cst.tile([128, 128], F32, tag="ident", name="ident")
            make_identity(nc, ident)
            ones_f = cst.tile([1, 128], F32, tag="ones_f", name="ones_f")
            nc.gpsimd.memset(ones_f[:], 1.0)
            ones1 = cst.tile([1, 128], BF16, tag="ones1", name="ones1")
            nc.vector.tensor_copy(ones1[:], ones_f[:])

            def load_const(src, shape, tag, dt=F32):
                t = cst.tile(shape, dt, tag=tag, name=tag)
                nc.sync.dma_start(t[:], src)
                return t

            textT_sb = load_const(textT[:], [T, B], "textT", BF16)
            wtT_sb = load_const(wtT[:], [T, D], "wtT", BF16)
            waT_sb = [load_const(waT[k * 128:(k + 1) * 128, :], [128, A],
                                 f"waT{k}", BF16) for k in range(DT)]
            wbT_sb = [load_const(wbT[k * 128:(k + 1) * 128, :], [128, A],
                                 f"wbT{k}", BF16) for k in range(DT)]
            wcT_sb = [load_const(wcT[k * 128:(k + 1) * 128, :], [128, 1],
                                 f"wcT{k}", BF16) for k in range(AT)]
            wh1T_sb = [load_const(wh1T[k * 128:(k + 1) * 128, :], [128, D],
                                  f"wh1T{k}", BF16) for k in range(DT)]
            wo1T_sb = [load_const(wo1T[k * 128:(k + 1) * 128, :], [128, D],
                                  f"wo1T{k}", BF16) for k in range(DT)]
            wclsT_sb = load_const(wclsT[:], [OS, 1], "wclsT", BF16)
            bic_sb = [load_const(b_ic[k * 128:(k + 1) * 128, :], [128, 1], f"bic{k}")
                      for k in range(DT)]
            ba_sb = [load_const(ba[k * 128:(k + 1) * 128, :], [128, 1], f"ba{k}")
                     for k in range(AT)]
            bb_sb = [load_const(bb[k * 128:(k + 1) * 128, :], [128, 1], f"bb{k}")
                     for k in range(AT)]
            bt_sb = [load_const(bt[k * 128:(k + 1) * 128, :], [128, 1], f"bt{k}")
                     for k in range(DT)]
            bh1_sb = [load_const(bh1[k * 128:(k + 1) * 128, :], [128, 1], f"bh1{k}")
                      for k in range(DT)]
            bo1_sb = [load_const(bo1[k * 128:(k + 1) * 128, :], [128, 1], f"bo1{k}")
                      for k in range(DT)]
            bz1_sb = load_const(bz1[:], [OS, 1], "bz1")
            bpf_sb = load_const(bpf[:], [OS, 1], "bpf")

            def emit_body(rep):
                # ---- big-weight slab FIFO: wz1[0..63] then wpf[0..63] ----
                slab_tiles = [None] * (2 * OS)
                state = {"next": 0}

                def emit_slabs(upto):
                    while state["next"] < min(upto, 2 * OS):
                        j = state["next"]
                        src = wz1[j] if j < OS else wpf[j - OS]
                        t = wbig.tile([128, DT * D], BF16, tag="slab",
                                      name=f"slab{rep}_{j}")
                        nc.sync.dma_start(t[:], src)
                        slab_tiles[j] = t
                        state["next"] = j + 1

                # ---- text branch: textc = relu(Wt @ text.T + bt) [D, B] ------
                textc = []
                for m in range(DT):
                    p = ps.tile([128, B], F32, tag="ps", name=f"tc_ps{m}")
                    nc.tensor.matmul(p[:], wtT_sb[:, m * 128:(m + 1) * 128],
                                     textT_sb[:], start=True, stop=True)
                    t = per.tile([128, B], F32, tag=f"textc{m}", name=f"textc{m}")
                    nc.scalar.activation(t[:], p[:],
                                         mybir.ActivationFunctionType.Relu,
                                         bias=bt_sb[m][:])
                    textc.append(t)
                # textcT [B, D] via PE transpose
                textcT = per.tile([B, D], F32, tag="textcT", name="textcT")
                for m in range(DT):
                    p = ps.tile([B, 128], F32, tag="ps", name=f"tct_ps{m}")
                    nc.tensor.transpose(p[:], textc[m][:], ident[:, :])
                    nc.vector.tensor_copy(textcT[:, m * 128:(m + 1) * 128], p[:])

                # ---- stage 1: partial img.T = (W_ic @ X.T) slice --------------
                # accumulate [D, BN] in 8 PSUM banks over 64 k-tiles;
                # interleave the first NSLAB wz1 slab loads for prefetch.
                s1ps = [ps.tile([128, 512], F32, tag="ps", name=f"s1ps{i}")
                        for i in range(8)]
                for c in range(KC):
                    xt = xk.tile([128, 2 * BN], BF16, tag="xk", name=f"x{c}")
                    nc.sync.dma_start(xt[:], xT[c])
                    wt = wk.tile([128, 2 * D], BF16, tag="wk", name=f"w{c}")
                    nc.sync.dma_start(wt[:], wicT[c])
                    if c < NSLAB:
                        emit_slabs(c + 1)
                    for k in range(2):
                        first = c == 0 and k == 0
                        last = c == KC - 1 and k == 1
                        for dt in range(DT):
                            for h in range(2):
                                nc.tensor.matmul(
                                    s1ps[dt * 2 + h][:],
                                    wt[:, k * D + dt * 128:k * D + (dt + 1) * 128],
                                    xt[:, k * BN + h * 512:k * BN + (h + 1) * 512],
                                    start=first, stop=last)

                # partial -> DRAM bounce -> AllReduce (bf16) -> img tiles
                shared_addr = "Shared" if use_collectives else "Local"
                ar_in = dp.tile([D, BN], BF16, tag="ar_in", name="ar_in")
                ar_out = dp.tile([D, BN], BF16, tag="ar_out", name="ar_out",
                                 addr_space=shared_addr)
                for dt in range(DT):
                    for h in range(2):
                        s = wrk.tile([128, 512], BF16, tag="s1out", bufs=2,
                                     name=f"s1o{dt}{h}")
                        nc.scalar.copy(s[:], s1ps[dt * 2 + h][:])
                        nc.sync.dma_start(
                            ar_in[dt * 128:(dt + 1) * 128, h * 512:(h + 1) * 512],
                            s[:])
                if use_collectives:
                    nc.gpsimd.collective_compute(
                        "AllReduce", mybir.AluOpType.add, replica_groups=groups,
                        ins=[ar_in.opt()], outs=[ar_out.opt()])
                else:
                    nc.sync.dma_start(ar_out[:], ar_in[:])

                # img = relu(sum + b_ic): [D, BN] as 4 tiles [128, BN]
                img = []
                for dt in range(DT):
                    raw = wrk.tile([128, BN], BF16, tag="imgraw", bufs=2,
                                   name=f"imgraw{dt}")
                    nc.sync.dma_start(raw[:], ar_out[dt * 128:(dt + 1) * 128, :])
                    t = per.tile([128, BN], BF16, tag=f"img{dt}", name=f"img{dt}")
                    nc.scalar.activation(t[:], raw[:],
                                         mybir.ActivationFunctionType.Relu,
                                         bias=bic_sb[dt][:])
                    img.append(t)

                # ---- attention: a=tanh(Wa@img+ba), g=sig(Wb@img+bb) ----------
                def attn_half(wT_sb, b_sb, func, nm):
                    outs = []
                    for m in range(AT):
                        t = wrk.tile([128, BN], BF16, tag=f"{nm}{m}", name=f"{nm}{m}")
                        for h in range(2):
                            p = ps.tile([128, 512], F32, tag="ps",
                                        name=f"{nm}_ps{m}{h}")
                            for k in range(DT):
                                nc.tensor.matmul(
                                    p[:],
                                    wT_sb[k][:, m * 128:(m + 1) * 128],
                                    img[k][:, h * 512:(h + 1) * 512],
                                    start=(k == 0), stop=(k == DT - 1))
                            nc.scalar.activation(t[:, h * 512:(h + 1) * 512], p[:],
                                                 func, bias=b_sb[m][:])
                        outs.append(t)
                    return outs

                a_sb = attn_half(waT_sb, ba_sb, mybir.ActivationFunctionType.Tanh, "a")
                g_sb = attn_half(wbT_sb, bb_sb, mybir.ActivationFunctionType.Sigmoid, "g")
                for m in range(AT):
                    nc.vector.tensor_mul(a_sb[m][:], a_sb[m][:], g_sb[m][:])

                # logits [1, BN] = Wc @ (a*g)
                sm = wrk.tile([1, BN], F32, tag="sm", name="sm")
                for h in range(2):
                    p = ps.tile([1, 512], F32, tag="ps", name=f"lg{h}")
                    for k in range(AT):
                        nc.tensor.matmul(p[:], wcT_sb[k][:],
                                         a_sb[k][:, h * 512:(h + 1) * 512],
                                         start=(k == 0), stop=(k == AT - 1))
                    nc.scalar.copy(sm[:, h * 512:(h + 1) * 512], p[:])

                # softmax over n (64) within each bag, * 1/N   -> wv [1, BN]
                smv = sm.rearrange("p (b n) -> p b n", n=N)
                mx = wrk.tile([1, B], F32, tag="mx", name="mx")
                nc.vector.tensor_reduce(mx[:], smv, mybir.AxisListType.X,
                                        mybir.AluOpType.max)
                ex = wrk.tile([1, BN], F32, tag="ex", name="ex")
                exv = ex.rearrange("p (b n) -> p b n", n=N)
                nc.vector.tensor_sub(exv, smv, mx[:, :, None].broadcast_to([1, B, N]))
                nc.scalar.activation(ex[:], ex[:], mybir.ActivationFunctionType.Exp)
                sumx = wrk.tile([1, B], F32, tag="sumx", name="sumx")
                nc.vector.tensor_reduce(sumx[:], exv, mybir.AxisListType.X,
                                        mybir.AluOpType.add)
                rc = wrk.tile([1, B], F32, tag="rc", name="rc")
                nc.vector.reciprocal(rc[:], sumx[:])
                wv = wrk.tile([1, BN], BF16, tag="wv", name="wv")
                nc.vector.scalar_tensor_tensor(
                    wv.rearrange("p (b n) -> p b n", n=N), exv, 1.0 / N,
                    rc[:, :, None].broadcast_to([1, B, N]),
                    op0=mybir.AluOpType.mult, op1=mybir.AluOpType.mult)

                # broadcast wv across partitions via K=1 matmul, then pool:
                # imgp[d, b] = sum_n img[d, (b,n)] * w[b, n]
                wb_ps = []
                for h in range(2):
                    p = ps.tile([128, 512], F32, tag="ps", name=f"wb_ps{h}")
                    nc.tensor.matmul(p[:], ones1[:],
                                     wv[:, h * 512:(h + 1) * 512],
                                     start=True, stop=True)
                    wb_ps.append(p)
                imgp = []
                for dt in range(DT):
                    scr = wrk.tile([128, BN], F32, tag="poolscr", bufs=1,
                                   name=f"pscr{dt}")
                    for h in range(2):
                        nc.vector.tensor_mul(scr[:, h * 512:(h + 1) * 512],
                                             img[dt][:, h * 512:(h + 1) * 512],
                                             wb_ps[h][:])
                    t = per.tile([128, B], BF16, tag=f"imgp{dt}", name=f"imgp{dt}")
                    nc.vector.tensor_reduce(t[:],
                                            scr.rearrange("p (b n) -> p b n", n=N),
                                            mybir.AxisListType.X,
                                            mybir.AluOpType.add)
                    imgp.append(t)

                # ---- h1 = relu(Wh1 @ imgp + bh1) [D, B] ----------------------
                h1 = []
                for m in range(DT):
                    p = ps.tile([128, B], F32, tag="ps", name=f"h1ps{m}")
                    for k in range(DT):
                        nc.tensor.matmul(p[:],
                                         wh1T_sb[k][:, m * 128:(m + 1) * 128],
                                         imgp[k][:],
                                         start=(k == 0), stop=(k == DT - 1))
                    t = per.tile([128, B], F32, tag=f"h1{m}", name=f"h1{m}")
                    nc.scalar.activation(t[:], p[:],
                                         mybir.ActivationFunctionType.Relu,
                                         bias=bh1_sb[m][:])
                    h1.append(t)

                # ---- bilinear + post-fusion share this pattern ----------------
                def big_bilinear(base, stat, outT_name):
                    """outT[b, o] = sum_ij stat[i, b] * w[o, i, j] * textcT[b, j]"""
                    outT = wrk.tile([B, OS], F32, tag=outT_name, name=outT_name)
                    for o in range(OS):
                        emit_slabs(base + o + NSLAB + 1)
                        wt = slab_tiles[base + o]
                        p = ps.tile([B, D], F32, tag="ps", name=f"{outT_name}_ps{o}")
                        for it in range(DT):
                            nc.tensor.matmul(p[:], stat[it][:],
                                             wt[:, it * D:(it + 1) * D],
                                             start=(it == 0), stop=(it == DT - 1))
                        scr = wrk.tile([B, D], F32, tag="ttr_scr", bufs=2,
                                       name=f"{outT_name}_scr{o}")
                        nc.vector.scalar_tensor_tensor(
                            scr[:], p[:], 1.0, textcT[:],
                            op0=mybir.AluOpType.mult, op1=mybir.AluOpType.mult,
                            accum_out=outT[:, o:o + 1])
                    return outT

                z1T = big_bilinear(0, imgp, "z1T")
                # sg_slice = sigmoid(z1 + bz1) [OS, B]; AllGather -> sg [D, B]
                zp = ps.tile([OS, B], F32, tag="ps", name="zp")
                nc.tensor.transpose(zp[:], z1T[:], ident[:B, :B])
                sg_sl = wrk.tile([OS, B], BF16, tag="sg_sl", name="sg_sl")
                nc.scalar.activation(sg_sl[:], zp[:],
                                     mybir.ActivationFunctionType.Sigmoid,
                                     bias=bz1_sb[:])
                ag_in = dp.tile([OS, B], BF16, tag="ag_in", name="ag_in")
                ag_out = dp.tile([D, B], BF16, tag="ag_out", name="ag_out",
                                 addr_space=shared_addr)
                nc.sync.dma_start(ag_in[:], sg_sl[:])
                if use_collectives:
                    nc.gpsimd.collective_compute(
                        "AllGather", mybir.AluOpType.bypass, replica_groups=groups,
                        ins=[ag_in.opt()], outs=[ag_out.opt()])
                else:
                    for r in range(NCORES):
                        nc.sync.dma_start(ag_out[r * OS:(r + 1) * OS, :], ag_in[:])

                # gate = sg * h1 ; o1 = relu(Wo1 @ gate + bo1) [D, B]
                o1 = []
                gate = []
                for dt in range(DT):
                    g = wrk.tile([128, B], BF16, tag=f"gate{dt}", name=f"gate{dt}")
                    nc.sync.dma_start(g[:], ag_out[dt * 128:(dt + 1) * 128, :])
                    nc.vector.tensor_mul(g[:], g[:], h1[dt][:])
                    gate.append(g)
                for m in range(DT):
                    p = ps.tile([128, B], F32, tag="ps", name=f"o1ps{m}")
                    for k in range(DT):
                        nc.tensor.matmul(p[:],
                                         wo1T_sb[k][:, m * 128:(m + 1) * 128],
                                         gate[k][:],
                                         start=(k == 0), stop=(k == DT - 1))
                    t = per.tile([128, B], BF16, tag=f"o1_{m}", name=f"o1_{m}")
                    nc.scalar.activation(t[:], p[:],
                                         mybir.ActivationFunctionType.Relu,
                                         bias=bo1_sb[m][:])
                    o1.append(t)

                # ---- post-fusion + classifier partial -------------------------
                fT = big_bilinear(OS, o1, "fT")
                fp = ps.tile([OS, B], F32, tag="ps", name="fp")
                nc.tensor.transpose(fp[:], fT[:], ident[:B, :B])
                fr = wrk.tile([OS, B], BF16, tag="fr", name="fr")
                nc.scalar.activation(fr[:], fp[:],
                                     mybir.ActivationFunctionType.Relu,
                                     bias=bpf_sb[:])
                cp = ps.tile([1, B], F32, tag="ps", name="cp")
                nc.tensor.matmul(cp[:], wclsT_sb[:], fr[:], start=True, stop=True)
                osb = wrk.tile([1, B], F32, tag="osb", name="osb")
                nc.vector.tensor_copy(osb[:], cp[:])
                nc.sync.dma_start(out_partial[:], osb[:])

            if reps > 1 and not use_collectives:
                with tc.For_i(0, reps, 1):
                    emit_body(0)
            else:
                for r in range(reps):
                    emit_body(r)

    nc.compile()
    return nc


_NC_CACHE = {}


def _get_nc():
    if "nc" not in _NC_CACHE:
        _NC_CACHE["nc"] = build_nc()
    return _NC_CACHE["nc"]


def _bf(a):
    return np.ascontiguousarray(np.asarray(a, np.float32)).astype(NP_BF16)


def _ktile(a2d):
    """[FS, M] -> [KC, 128, 2*M] matching the kernel's (c, p, k*M+m) layout."""
    fs, m = a2d.shape
    return np.ascontiguousarray(
        a2d.reshape(KC, 2, 128, m).transpose(0, 2, 1, 3).reshape(KC, 128, 2 * m))


def _slab(a3d):
    """[OS, D, D] -> [OS, 128, DT*D] matching (o, p, it*D+j)."""
    return np.ascontiguousarray(
        a3d.reshape(OS, DT, 128, D).transpose(0, 2, 1, 3).reshape(OS, 128, DT * D))


def make_in_maps(inputs):
    """Host-side sharding of the full inputs into 8 per-core input maps."""
    ii = {k: np.asarray(v, dtype=np.float32) for k, v in inputs.items()}
    X = ii["image_features"].reshape(BN, F)
    wz1_full = ii["Wz1"]
    wpf_full = ii["Wpf"].reshape(D, D, D)

    shared = {
        "textT": _bf(ii["text_features"].T),
        "waT": _bf(ii["Wa"].T),
        "wbT": _bf(ii["Wb"].T),
        "wcT": _bf(ii["Wc"].T),
        "wtT": _bf(ii["Wt"].T),
        "wh1T": _bf(ii["Wh1"].T),
        "wo1T": _bf(ii["Wo1"].T),
        "b_ic": ii["b_ic"].reshape(D, 1),
        "ba": ii["ba"].reshape(A, 1),
        "bb": ii["bb"].reshape(A, 1),
        "bt": ii["bt"].reshape(D, 1),
        "bh1": ii["bh1"].reshape(D, 1),
        "bo1": ii["bo1"].reshape(D, 1),
    }
    in_maps = []
    for c in range(NCORES):
        fs = slice(c * FS, (c + 1) * FS)
        os_ = slice(c * OS, (c + 1) * OS)
        m = dict(shared)
        m["xT"] = _ktile(_bf(X[:, fs].T))
        m["wicT"] = _ktile(_bf(ii["W_ic"][:, fs].T))
        m["wz1"] = _slab(_bf(wz1_full[os_]))
        m["wpf"] = _slab(_bf(wpf_full[os_]))
        m["wclsT"] = _bf(ii["Wcls"][0, os_].reshape(OS, 1))
        m["bz1"] = np.ascontiguousarray(ii["bz1"][os_].reshape(OS, 1))
        m["bpf"] = np.ascontiguousarray(ii["bpf"][os_].reshape(OS, 1))
        in_maps.append(m)
    return in_maps


def gather_output(results, bcls):
    acc = np.zeros((1, B), np.float64)
    for c in range(NCORES):
        acc += results[c]["out_partial"].astype(np.float64)
    return (acc.T + bcls.astype(np.float64)).astype(np.float32)


def kernel(**inputs) -> np.ndarray:
    from concourse.bass_utils import run_bass_kernel_spmd

    nc = _get_nc()
    in_maps = make_in_maps(inputs)
    res = run_bass_kernel_spmd(nc, in_maps, list(range(NCORES)))
    return gather_output(res.results, np.asarray(inputs["bcls"], np.float32))


# revision 25
# speedup vs baseline: 1.1991x; 1.1991x over previous
"""Trainium2 Bass kernel for ConcreteAttentionModel (dense_mlp).

Model (reference):
  img = relu(einsum('bnf,df->bnd', image_features, W_ic) + b_ic)   B=16,N=64,F=65536,D=512
  gated attention-MIL pooling over patches -> img [B, D]
  text = relu(text @ Wt.T + bt)                                    [B, D]
  h1 = relu(img @ Wh1.T + bh1)
  z1 = einsum('bi,oij,bj->bo', img, Wz1, text) + bz1               Wz1 [D,D,D]
  o1 = relu((sigmoid(z1) * h1) @ Wo1.T + bo1)
  fused = relu(kron(o1, text) @ Wpf.T + bpf)                       Wpf [D, D*D]
  out = fused @ Wcls.T + bcls                                      [B, 1]

Sharding over 8 cores (v4: fp8 stage-1 + fp8 bilinear, bf16 elsewhere):
  - Stage 1 sharded over the contraction dim F (8192 per core).  X and
    64*W_ic are fp8e4m3; matmuls run in DoubleRow mode (K=256 per mm).
    bf16 partial [D, BN] AllReduce'd on-device; relu applies scale=1/64.
    Attention pooling replicated per core.
  - Bilinear Wz1 (fp8, x64, DoubleRow) and post-fusion Wpf (bf16) sharded
    over their output dim (64 rows per core).  Weight slabs stream through
    two FIFO pools sized so prefetch fills the PE-bound stage-1 window.
    Outputs are packed 4-per-PSUM-bank (32-aligned) so the j-contraction
    against text runs as one DVE pass per 4 rows.
  - Latency-critical small DMAs go on the scalar (ACT) HWDGE ring so they
    never queue behind slab streams on the sync ring.
"""

import numpy as np
import ml_dtypes

import concourse.bass as bass
import concourse.mybir as mybir
import concourse.tile as tile
from concourse import bacc
from concourse.masks import make_identity

F32 = mybir.dt.float32
BF16 = mybir.dt.bfloat16
F8 = mybir.dt.float8e4
NP_BF16 = ml_dtypes.bfloat16
NP_F8 = mybir.dt.np(F8)
DR = mybir.MatmulPerfMode.DoubleRow

NCORES = 8
B, N, F = 16, 64, 65536
BN = B * N                      # 1024
D, A, T = 512, 256, 18
FS = F // NCORES                # 8192 per-core contraction slice
KT = FS // 128                  # 64 k-tiles
KC = KT // 2                    # 32 k-chunks (2 k-tiles per DMA / DoubleRow mm)
OS = D // NCORES                # 64 output rows per core (bilinear/post-fusion)
DT = D // 128                   # 4 partition tiles of the D dim
AT = A // 128                   # 2 partition tiles of the A dim
WSCALE = 64.0                   # fp8 weight pre-scale (avoids subnormals)
WZ_WIN = 32                     # in-flight wz1 fp8 slabs
WPF_WIN = 15                    # in-flight wpf bf16 slabs
NB = OS // 4                    # 16 bilinear batches of 4 output rows


def build_nc(use_collectives: bool = True, reps: int = 1):
    nc = bacc.Bacc("TRN2", target_bir_lowering=False, debug=False,
                   num_devices=NCORES)

    def dram(name, shape, dt=F32):
        return nc.dram_tensor(name, shape, dt, kind="ExternalInput").ap()

    # per-core sharded inputs, pre-tiled host-side for contiguous DMA
    xT = dram("xT", [KC, 128, 2 * BN], F8)      # [c, p, k*BN+n]
    wicT = dram("wicT", [KC, 128, 2 * D], F8)   # [c, p, k*D+d], x64
    wz1 = dram("wz1", [OS, 128, DT * D], F8)    # [o, p, it*D+j], x64
    wpf = dram("wpf", [OS, 128, DT * D], BF16)  # [m, p, it*D+j]
    wclsT = dram("wclsT", [OS, 1], BF16)
    bz1 = dram("bz1", [OS, 1])
    bpf = dram("bpf", [OS, 1])
    # replicated inputs
    textT = dram("textT", [T, B], BF16)
    waT = dram("waT", [D, A], BF16)
    wbT = dram("wbT", [D, A], BF16)
    wcT = dram("wcT", [A, 1], BF16)
    wtT = dram("wtT", [T, D], BF16)
    wh1T = dram("wh1T", [D, D], BF16)
    wo1T = dram("wo1T", [D, D], BF16)
    b_ic = dram("b_ic", [D, 1])
    ba = dram("ba", [A, 1])
    bb = dram("bb", [A, 1])
    bt = dram("bt", [D, 1])
    bh1 = dram("bh1", [D, 1])
    bo1 = dram("bo1", [D, 1])

    out_partial = nc.dram_tensor("out_partial", [1, B], F32,
                                 kind="ExternalOutput").ap()

    groups = [list(range(NCORES))]

    with tile.TileContext(nc) as tc:
        with (
            nc.allow_low_precision(reason="fp8/bf16 matmul operands"),
            tc.tile_pool(name="const", bufs=1) as cst,
            tc.tile_pool(name="xk", bufs=6) as xk,
            tc.tile_pool(name="wk", bufs=6) as wk,
            tc.tile_pool(name="wzp", bufs=WZ_WIN) as wzp,
            tc.tile_pool(name="wpfp", bufs=WPF_WIN) as wpfp,
            tc.tile_pool(name="work", bufs=1) as wrk,
            tc.tile_pool(name="persist", bufs=1) as per,
            tc.tile_pool(name="ps", bufs=8, space="PSUM") as ps,
            tc.tile_pool(name="dramp", bufs=1, space="DRAM") as dp,
        ):
            # ---- constants / small weights (scalar ring: keep sync free) --
            ident = cst.tile([128, 128], F32, tag="ident", name="ident")
            make_identity(nc, ident)
            ones_f = cst.tile([1, 128], F32, tag="ones_f", name="ones_f")
            nc.gpsimd.memset(ones_f[:], 1.0)
            ones1 = cst.tile([1, 128], BF16, tag="ones1", name="ones1")
            nc.vector.tensor_copy(ones1[:], ones_f[:])
            def load_const(src, shape, tag, dt=F32):
                t = cst.tile(shape, dt, tag=tag, name=tag)
                nc.scalar.dma_start(t[:], src)
                return t

            textT_sb = load_const(textT[:], [T, B], "textT", BF16)
            wtT_sb = load_const(wtT[:], [T, D], "wtT", BF16)
            waT_sb = [load_const(waT[k * 128:(k + 1) * 128, :], [128, A],
                                 f"waT{k}", BF16) for k in range(DT)]
            wbT_sb = [load_const(wbT[k * 128:(k + 1) * 128, :], [128, A],
                                 f"wbT{k}", BF16) for k in range(DT)]
            wcT_sb = [load_const(wcT[k * 128:(k + 1) * 128, :], [128, 1],
                                 f"wcT{k}", BF16) for k in range(AT)]
            wh1T_sb = [load_const(wh1T[k * 128:(k + 1) * 128, :], [128, D],
                                  f"wh1T{k}", BF16) for k in range(DT)]
            wo1T_sb = [load_const(wo1T[k * 128:(k + 1) * 128, :], [128, D],
                                  f"wo1T{k}", BF16) for k in range(DT)]
            wclsT_sb = load_const(wclsT[:], [OS, 1], "wclsT", BF16)
            bic_sb = [load_const(b_ic[k * 128:(k + 1) * 128, :], [128, 1], f"bic{k}")
                      for k in range(DT)]
            ba_sb = [load_const(ba[k * 128:(k + 1) * 128, :], [128, 1], f"ba{k}")
                     for k in range(AT)]
            bb_sb = [load_const(bb[k * 128:(k + 1) * 128, :], [128, 1], f"bb{k}")
                     for k in range(AT)]
            bt_sb = [load_const(bt[k * 128:(k + 1) * 128, :], [128, 1], f"bt{k}")
                     for k in range(DT)]
            bh1_sb = [load_const(bh1[k * 128:(k + 1) * 128, :], [128, 1], f"bh1{k}")
                      for k in range(DT)]
            bo1_sb = [load_const(bo1[k * 128:(k + 1) * 128, :], [128, 1], f"bo1{k}")
                      for k in range(DT)]
            bz1_sb = load_const(bz1[:], [OS, 1], "bz1")
            bpf_sb = load_const(bpf[:], [OS, 1], "bpf")

            def emit_body(rep):
                # ---- weight-slab FIFOs (emission order == consumption) ----
                # batch g consumes rows {q*16+g : q}, so emission is strided
                wz_tiles = [None] * OS   # indexed by emission order j
                wpf_tiles = [None] * OS
                st = {"wz": 0, "wpf": 0}

                def wz_emit(upto):
                    while st["wz"] < min(upto, OS):
                        j = st["wz"]
                        t = wzp.tile([128, DT, D], F8, tag="wz",
                                     name=f"wz{rep}_{j}")
                        nc.sync.dma_start(
                            t[:], wz1[j].rearrange("p (it j) -> p it j", it=DT))
                        wz_tiles[j] = t
                        st["wz"] = j + 1

                def wpf_emit(upto):
                    while st["wpf"] < min(upto, OS):
                        j = st["wpf"]
                        t = wpfp.tile([128, DT, D], BF16, tag="wpf",
                                      name=f"wpf{rep}_{j}")
                        nc.sync.dma_start(
                            t[:], wpf[j].rearrange("p (it j) -> p it j", it=DT))
                        wpf_tiles[j] = t
                        st["wpf"] = j + 1

                # ---- stage 1: partial img.T = (64*W_ic @ X.T), fp8 DoubleRow --
                s1ps = [ps.tile([128, 512], F32, tag="ps", name=f"s1ps{i}")
                        for i in range(8)]
                for c in range(KC):
                    xt = xk.tile([128, 2, BN], F8, tag="xk", name=f"x{c}")
                    nc.sync.dma_start(
                        xt[:], xT[c].rearrange("p (k n) -> p k n", k=2))
                    wt = wk.tile([128, 2, D], F8, tag="wk", name=f"w{c}")
                    nc.sync.dma_start(
                        wt[:], wicT[c].rearrange("p (k d) -> p k d", k=2))
                    wz_emit(c + 1)
                    if c >= KC - WPF_WIN:
                        wpf_emit(c - (KC - WPF_WIN) + 1)
                    for dt in range(DT):
                        for h in range(2):
                            nc.tensor.matmul(
                                s1ps[dt * 2 + h][:],
                                wt[:, :, dt * 128:(dt + 1) * 128],
                                xt[:, :, h * 512:(h + 1) * 512],
                                start=(c == 0), stop=(c == KC - 1),
                                perf_mode=DR)

                # partial -> DRAM bounce -> AllReduce (bf16) -> img tiles
                # (pairs of PSUM banks batched into one store, both HWDGE rings)
                shared_addr = "Shared" if use_collectives else "Local"
                ar_in = dp.tile([D, BN], BF16, tag="ar_in", name="ar_in")
                ar_out = dp.tile([D, BN], BF16, tag="ar_out", name="ar_out",
                                 addr_space=shared_addr)
                for dt in range(DT):
                    s = wrk.tile([128, BN], BF16, tag="s1out", bufs=2,
                                 name=f"s1o{dt}")
                    for h in range(2):
                        nc.scalar.copy(s[:, h * 512:(h + 1) * 512],
                                       s1ps[dt * 2 + h][:])
                    eng = nc.sync if dt % 2 == 0 else nc.scalar
                    eng.dma_start(ar_in[dt * 128:(dt + 1) * 128, :], s[:])

                # ---- text branch (runs in the AllReduce window) --------------
                textc = []
                for m in range(DT):
                    p = ps.tile([128, B], F32, tag="ps", name=f"tc_ps{m}")
                    nc.tensor.matmul(p[:], wtT_sb[:, m * 128:(m + 1) * 128],
                                     textT_sb[:], start=True, stop=True)
                    t = per.tile([128, B], F32, tag=f"textc{m}", name=f"textc{m}")
                    nc.scalar.activation(t[:], p[:],
                                         mybir.ActivationFunctionType.Relu,
                                         bias=bt_sb[m][:])
                    textc.append(t)
                # textcT [B, D] bf16 via PE transpose; then replicate to 128 rows
                textcT = per.tile([B, D], BF16, tag="textcT", name="textcT")
                for m in range(DT):
                    p = ps.tile([B, 128], F32, tag="ps", name=f"tct_ps{m}")
                    nc.tensor.transpose(p[:], textc[m][:], ident[:, :])
                    nc.vector.tensor_copy(textcT[:, m * 128:(m + 1) * 128], p[:])

                if use_collectives:
                    nc.gpsimd.collective_compute(
                        "AllReduce", mybir.AluOpType.add, replica_groups=groups,
                        ins=[ar_in.opt()], outs=[ar_out.opt()])
                else:
                    nc.sync.dma_start(ar_out[:], ar_in[:])

                # img = relu(sum/64 + b_ic): [D, BN] as 4 tiles [128, BN]
                img = []
                for dt in range(DT):
                    raw = wrk.tile([128, BN], BF16, tag="imgraw", bufs=2,
                                   name=f"imgraw{dt}")
                    nc.scalar.dma_start(raw[:],
                                        ar_out[dt * 128:(dt + 1) * 128, :])
                    t = per.tile([128, BN], BF16, tag=f"img{dt}", name=f"img{dt}")
                    nc.scalar.activation(t[:], raw[:],
                                         mybir.ActivationFunctionType.Relu,
                                         bias=bic_sb[dt][:], scale=1.0 / WSCALE)
                    img.append(t)

                # ---- attention: a=tanh(Wa@img+ba), g=sig(Wb@img+bb) ----------
                def attn_half(wT_sb, b_sb, func, nm):
                    outs = []
                    for m in range(AT):
                        t = wrk.tile([128, BN], BF16, tag=f"{nm}{m}", name=f"{nm}{m}")
                        for h in range(2):
                            p = ps.tile([128, 512], F32, tag="ps",
                                        name=f"{nm}_ps{m}{h}")
                            for k in range(DT):
                                nc.tensor.matmul(
                                    p[:],
                                    wT_sb[k][:, m * 128:(m + 1) * 128],
                                    img[k][:, h * 512:(h + 1) * 512],
                                    start=(k == 0), stop=(k == DT - 1))
                            nc.scalar.activation(t[:, h * 512:(h + 1) * 512], p[:],
                                                 func, bias=b_sb[m][:])
                        outs.append(t)
                    return outs

                a_sb = attn_half(waT_sb, ba_sb, mybir.ActivationFunctionType.Tanh, "a")
                g_sb = attn_half(wbT_sb, bb_sb, mybir.ActivationFunctionType.Sigmoid, "g")
                for m in range(AT):
                    nc.vector.tensor_mul(a_sb[m][:], a_sb[m][:], g_sb[m][:])

                # logits [1, BN] = Wc @ (a*g)
                sm = wrk.tile([1, BN], F32, tag="sm", name="sm")
                for h in range(2):
                    p = ps.tile([1, 512], F32, tag="ps", name=f"lg{h}")
                    for k in range(AT):
                        nc.tensor.matmul(p[:], wcT_sb[k][:],
                                         a_sb[k][:, h * 512:(h + 1) * 512],
                                         start=(k == 0), stop=(k == AT - 1))
                    nc.scalar.copy(sm[:, h * 512:(h + 1) * 512], p[:])

                # softmax over n (64) within each bag, * 1/N   -> wv [1, BN]
                smv = sm.rearrange("p (b n) -> p b n", n=N)
                mx = wrk.tile([1, B], F32, tag="mx", name="mx")
                nc.vector.tensor_reduce(mx[:], smv, mybir.AxisListType.X,
                                        mybir.AluOpType.max)
                ex = wrk.tile([1, BN], F32, tag="ex", name="ex")
                exv = ex.rearrange("p (b n) -> p b n", n=N)
                nc.vector.tensor_sub(exv, smv, mx[:, :, None].broadcast_to([1, B, N]))
                nc.scalar.activation(ex[:], ex[:], mybir.ActivationFunctionType.Exp)
                sumx = wrk.tile([1, B], F32, tag="sumx", name="sumx")
                nc.vector.tensor_reduce(sumx[:], exv, mybir.AxisListType.X,
                                        mybir.AluOpType.add)
                rc = wrk.tile([1, B], F32, tag="rc", name="rc")
                nc.vector.reciprocal(rc[:], sumx[:])
                wv = wrk.tile([1, BN], BF16, tag="wv", name="wv")
                nc.vector.scalar_tensor_tensor(
                    wv.rearrange("p (b n) -> p b n", n=N), exv, 1.0 / N,
                    rc[:, :, None].broadcast_to([1, B, N]),
                    op0=mybir.AluOpType.mult, op1=mybir.AluOpType.mult)

                # broadcast wv across partitions via K=1 matmul, then pool:
                # imgp[d, b] = sum_n img[d, (b,n)] * w[b, n]
                wb_ps = []
                for h in range(2):
                    p = ps.tile([128, 512], F32, tag="ps", name=f"wb_ps{h}")
                    nc.tensor.matmul(p[:], ones1[:],
                                     wv[:, h * 512:(h + 1) * 512],
                                     start=True, stop=True)
                    wb_ps.append(p)
                imgp = []
                imgp8 = per.tile([128, DT, B], F8, tag="imgp8", name="imgp8")
                for dt in range(DT):
                    scr = wrk.tile([128, BN], F32, tag="poolscr", bufs=1,
                                   name=f"pscr{dt}")
                    for h in range(2):
                        nc.vector.tensor_mul(scr[:, h * 512:(h + 1) * 512],
                                             img[dt][:, h * 512:(h + 1) * 512],
                                             wb_ps[h][:])
                    t = per.tile([128, B], BF16, tag=f"imgp{dt}", name=f"imgp{dt}")
                    nc.vector.tensor_reduce(t[:],
                                            scr.rearrange("p (b n) -> p b n", n=N),
                                            mybir.AxisListType.X,
                                            mybir.AluOpType.add)
                    nc.vector.tensor_copy(imgp8[:, dt, :], t[:])
                    imgp.append(t)

                # ---- h1 = relu(Wh1 @ imgp + bh1) [D, B] ----------------------
                h1 = []
                for m in range(DT):
                    p = ps.tile([128, B], F32, tag="ps", name=f"h1ps{m}")
                    for k in range(DT):
                        nc.tensor.matmul(p[:],
                                         wh1T_sb[k][:, m * 128:(m + 1) * 128],
                                         imgp[k][:],
                                         start=(k == 0), stop=(k == DT - 1))
                    t = per.tile([128, B], F32, tag=f"h1{m}", name=f"h1{m}")
                    nc.scalar.activation(t[:], p[:],
                                         mybir.ActivationFunctionType.Relu,
                                         bias=bh1_sb[m][:])
                    h1.append(t)

                # ---- z1 bilinear: fp8 DoubleRow, one output row at a time ----
                z1T = wrk.tile([B, OS], F32, tag="z1T", name="z1T")
                for o in range(OS):
                    wz_emit(o + 1 + (WZ_WIN - 1))
                    wt = wz_tiles[o]
                    p = ps.tile([B, D], F32, tag="ps", name=f"z1ps{o}")
                    for i2 in range(2):
                        nc.tensor.matmul(
                            p[:], imgp8[:, 2 * i2:2 * i2 + 2, :],
                            wt[:, 2 * i2:2 * i2 + 2, :],
                            start=(i2 == 0), stop=(i2 == 1), perf_mode=DR)
                    scr = wrk.tile([B, D], F32, tag="ttr_scr", bufs=2,
                                   name=f"z1scr{o}")
                    nc.vector.scalar_tensor_tensor(
                        scr[:], p[:], 1.0, textcT[:],
                        op0=mybir.AluOpType.mult, op1=mybir.AluOpType.mult,
                        accum_out=z1T[:, o:o + 1])

                # sg = sigmoid(z1/64 + bz1) [OS, B]; AllGather -> [D, B]
                ag_in = dp.tile([OS, B], BF16, tag="ag_in", name="ag_in")
                ag_out = dp.tile([D, B], BF16, tag="ag_out", name="ag_out",
                                 addr_space=shared_addr)
                zp = ps.tile([OS, B], F32, tag="ps", name="zp")
                nc.tensor.transpose(zp[:], z1T[:], ident[:B, :B])
                sg_sl = wrk.tile([OS, B], BF16, tag="sg_sl", name="sg_sl")
                nc.scalar.activation(sg_sl[:], zp[:],
                                     mybir.ActivationFunctionType.Sigmoid,
                                     bias=bz1_sb[:], scale=1.0 / WSCALE)
                nc.scalar.dma_start(ag_in[:], sg_sl[:])
                if use_collectives:
                    nc.gpsimd.collective_compute(
                        "AllGather", mybir.AluOpType.bypass, replica_groups=groups,
                        ins=[ag_in.opt()], outs=[ag_out.opt()])
                else:
                    for r in range(NCORES):
                        nc.sync.dma_start(ag_out[r * OS:(r + 1) * OS, :], ag_in[:])

                # gate = sg * h1 ; o1 = relu(Wo1 @ gate + bo1) [D, B]
                o1 = []
                gate = []
                for dt in range(DT):
                    g_ = wrk.tile([128, B], BF16, tag=f"gate{dt}", name=f"gate{dt}")
                    nc.scalar.dma_start(g_[:], ag_out[dt * 128:(dt + 1) * 128, :])
                    nc.vector.tensor_mul(g_[:], g_[:], h1[dt][:])
                    gate.append(g_)
                for m in range(DT):
                    p = ps.tile([128, B], F32, tag="ps", name=f"o1ps{m}")
                    for k in range(DT):
                        nc.tensor.matmul(p[:],
                                         wo1T_sb[k][:, m * 128:(m + 1) * 128],
                                         gate[k][:],
                                         start=(k == 0), stop=(k == DT - 1))
                    t = per.tile([128, B], BF16, tag=f"o1_{m}", name=f"o1_{m}")
                    nc.scalar.activation(t[:], p[:],
                                         mybir.ActivationFunctionType.Relu,
                                         bias=bo1_sb[m][:])
                    o1.append(t)

                # ---- post-fusion (bf16) + classifier partial -----------------
                fT = wrk.tile([B, OS], F32, tag="fT", name="fT")
                for o in range(OS):
                    wpf_emit(o + 1 + (WPF_WIN - 1))
                    wt = wpf_tiles[o]
                    p = ps.tile([B, D], F32, tag="ps", name=f"pfps{o}")
                    for it in range(DT):
                        nc.tensor.matmul(p[:], o1[it][:], wt[:, it, :],
                                         start=(it == 0), stop=(it == DT - 1))
                    scr = wrk.tile([B, D], F32, tag="ttr_scr", bufs=2,
                                   name=f"pfscr{o}")
                    nc.vector.scalar_tensor_tensor(
                        scr[:], p[:], 1.0, textcT[:],
                        op0=mybir.AluOpType.mult, op1=mybir.AluOpType.mult,
                        accum_out=fT[:, o:o + 1])

                fp = ps.tile([OS, B], F32, tag="ps", name="fp")
                nc.tensor.transpose(fp[:], fT[:], ident[:B, :B])
                fr = wrk.tile([OS, B], BF16, tag="fr", name="fr")
                nc.scalar.activation(fr[:], fp[:],
                                     mybir.ActivationFunctionType.Relu,
                                     bias=bpf_sb[:])
                cp = ps.tile([1, B], F32, tag="ps", name="cp")
                nc.tensor.matmul(cp[:], wclsT_sb[:], fr[:], start=True, stop=True)
                osb = wrk.tile([1, B], F32, tag="osb", name="osb")
                nc.vector.tensor_copy(osb[:], cp[:])
                nc.scalar.dma_start(out_partial[:], osb[:])

            if reps > 1 and not use_collectives:
                with tc.For_i(0, reps, 1):
                    emit_body(0)
            else:
                for r in range(reps):
                    emit_body(r)

    nc.compile()
    return nc


_NC_CACHE = {}


def _get_nc():
    if "nc" not in _NC_CACHE:
        _NC_CACHE["nc"] = build_nc()
    return _NC_CACHE["nc"]


def _bf(a):
    return np.ascontiguousarray(np.asarray(a, np.float32)).astype(NP_BF16)


def _f8(a):
    return np.ascontiguousarray(np.asarray(a, np.float32)).astype(NP_F8)


def _ktile(a2d):
    """[FS, M] -> [KC, 128, 2*M] matching the kernel's (c, p, k*M+m) layout."""
    fs, m = a2d.shape
    return np.ascontiguousarray(
        a2d.reshape(KC, 2, 128, m).transpose(0, 2, 1, 3).reshape(KC, 128, 2 * m))


def _slab(a3d):
    """[OS, D, D] -> [OS, 128, DT*D] matching (o, p, it*D+j)."""
    return np.ascontiguousarray(
        a3d.reshape(OS, DT, 128, D).transpose(0, 2, 1, 3).reshape(OS, 128, DT * D))


def make_in_maps(inputs):
    """Host-side sharding of the full inputs into 8 per-core input maps."""
    ii = {k: np.asarray(v, dtype=np.float32) for k, v in inputs.items()}
    X = ii["image_features"].reshape(BN, F)
    wz1_full = ii["Wz1"]
    wpf_full = ii["Wpf"].reshape(D, D, D)

    shared = {
        "textT": _bf(ii["text_features"].T),
        "waT": _bf(ii["Wa"].T),
        "wbT": _bf(ii["Wb"].T),
        "wcT": _bf(ii["Wc"].T),
        "wtT": _bf(ii["Wt"].T),
        "wh1T": _bf(ii["Wh1"].T),
        "wo1T": _bf(ii["Wo1"].T),
        "b_ic": ii["b_ic"].reshape(D, 1),
        "ba": ii["ba"].reshape(A, 1),
        "bb": ii["bb"].reshape(A, 1),
        "bt": ii["bt"].reshape(D, 1),
        "bh1": ii["bh1"].reshape(D, 1),
        "bo1": ii["bo1"].reshape(D, 1),
    }
    in_maps = []
    for c in range(NCORES):
        fs = slice(c * FS, (c + 1) * FS)
        os_ = slice(c * OS, (c + 1) * OS)
        m = dict(shared)
        m["xT"] = _ktile(_f8(X[:, fs].T))
        m["wicT"] = _ktile(_f8(ii["W_ic"][:, fs].T * WSCALE))
        m["wz1"] = _slab(_f8(wz1_full[os_] * WSCALE))
        m["wpf"] = _slab(_bf(wpf_full[os_]))
        m["wclsT"] = _bf(ii["Wcls"][0, os_].reshape(OS, 1))
        m["bz1"] = np.ascontiguousarray(ii["bz1"][os_].reshape(OS, 1))
        m["bpf"] = np.ascontiguousarray(ii["bpf"][os_].reshape(OS, 1))
        in_maps.append(m)
    return in_maps


def gather_output(results, bcls):
    acc = np.zeros((1, B), np.float64)
    for c in range(NCORES):
        acc += results[c]["out_partial"].astype(np.float64)
    return (acc.T + bcls.astype(np.float64)).astype(np.float32)


def kernel(**inputs) -> np.ndarray:
    from concourse.bass_utils import run_bass_kernel_spmd

    nc = _get_nc()
    in_maps = make_in_maps(inputs)
    res = run_bass_kernel_spmd(nc, in_maps, list(range(NCORES)))
    return gather_output(res.results, np.asarray(inputs["bcls"], np.float32))
